# revision 2
# baseline (speedup 1.0000x reference)
"""NT-Xent (SimCLR) contrastive loss on 8 Trainium2 NeuronCores.

Strategy (row-sharded similarity matrix):
  Z = concat(z_i, z_j) -> [N=8192, D=256].  Every core receives the full
  z_i / z_j plus its own raw 1024-row slab of Z.  On device, each core
  - casts its slab to bf16 (raw, unnormalized) and stages it through DRAM,
    reading it back with the DMA xbar transpose to get the stationary
    operand [D, 1024]; the slab's 1/|z| folds into the Exp scale later,
  - L2-normalizes all N rows (DVE square+accum, bit-trick rsqrt), casts to
    bf16, stages to DRAM and xbar-transposes back as the moving operand,
  - computes its [1024, 8192] slab of logits = (z_slab @ Zhat^T) via bf16
    matmuls (k-outer / j-inner so 4 consecutive matmuls share a stationary
    tile -> LDWEIGHTS is hoisted and matmuls pipeline),
  - applies exp(inv_norm_m * logits / T) on ScalarE with fused
    per-partition accumulation (accum_out) to produce row sums directly,
  - DMAs out one [128, 8] f32 tile of slab row sums.
  DMA queues: raw loads + staging stores ride the gpsimd SWDGE queue and
  the xbar transposes ride the sync HWDGE queue, so ScalarE's instruction
  stream is pure Exp (the EXP chain is the critical engine at ~64us).
  The host then computes loss = mean(log(rowsum - e^{1/T}) - pos/T).
"""

import math

import numpy as np

import concourse.bacc as bacc
import concourse.bass as bass
import concourse.mybir as mybir
import concourse.tile as tile
from concourse.bass_utils import run_bass_kernel_spmd

B, D = 4096, 256
N = 2 * B                      # 8192 rows of Z
N_CORES = 8
SLAB = N // N_CORES            # 1024 rows per core
TEMPERATURE = 0.5
INV_T = 1.0 / TEMPERATURE      # 2.0

F32 = mybir.dt.float32
BF16 = mybir.dt.bfloat16
I32 = mybir.dt.int32
ALU = mybir.AluOpType
ACT = mybir.ActivationFunctionType

GROUPS = N // SLAB             # 8 groups of 1024 rows (8x 128-row subtiles)
SUBT = SLAB // 128             # 8 subtiles per group
KT = D // 128                  # 2 contraction tiles
CHUNK = 512                    # matmul moving free dim / PSUM bank
CHUNKS = N // CHUNK            # 16 column chunks
JG = 4                         # chunks per PSUM tile ([128, 2048] = 4 banks)
NJG = CHUNKS // JG             # 4 chunk-groups
MT = SLAB // 128               # 8 output row tiles per core

RSQRT_MAGIC = 0x5F3759DF


def _emit_rsqrt(nc, pool, n2, inv, cols):
    """inv = 1/sqrt(n2), elementwise on a [128, cols] f32 tile.

    Quake-style exponent-halving seed + 2 Newton-Raphson steps, all on DVE
    (ScalarE's Rsqrt table is banned for accuracy; keeping ScalarE free for
    the hot-loop Exp)."""
    t_int = pool.tile([128, cols], I32, tag="rsq_i")
    y = pool.tile([128, cols], F32, tag="rsq_y")
    a = pool.tile([128, cols], F32, tag="rsq_a")
    c = pool.tile([128, cols], F32, tag="rsq_c")
    # y0 = bits^-1(MAGIC - bits(n2) >> 1)  (can't mix bitwise+arith in one op)
    nc.vector.tensor_scalar(
        t_int[:], n2.bitcast(I32), 1, None, op0=ALU.logical_shift_right)
    nc.vector.tensor_scalar(
        y.bitcast(I32), t_int[:], -1, RSQRT_MAGIC, op0=ALU.mult, op1=ALU.add)
    for it in range(2):  # y <- y * (1.5 - 0.5*n2*y^2); ~5e-6 max rel err
        nc.vector.scalar_tensor_tensor(
            a[:], y[:], 1.0, y[:], op0=ALU.bypass, op1=ALU.mult)
        nc.vector.scalar_tensor_tensor(
            c[:], a[:], -0.5, n2, op0=ALU.mult, op1=ALU.mult)
        nc.vector.scalar_tensor_tensor(
            inv if it == 1 else y[:], c[:], 1.5, y[:],
            op0=ALU.add, op1=ALU.mult)


def _emit_normalize_group(nc, pools, raw_src_ap, znorm_dram_ap, zt_dst_aps):
    """Load 1024 raw f32 rows, L2-normalize them, cast to bf16, stage to DRAM
    and xbar-transpose back into the [128, k, 1024] destination slices."""
    work, small = pools["work"], pools["small"]
    raw = work.tile([128, SUBT, D], F32, tag="raw")
    nc.gpsimd.dma_start(raw[:], raw_src_ap)

    sq_dump = work.tile([128, D], F32, tag="sqdump")
    n2 = small.tile([128, SUBT], F32, tag="n2")
    for t in range(SUBT):
        nc.vector.scalar_tensor_tensor(
            sq_dump[:], raw[:, t], 1.0, raw[:, t],
            op0=ALU.bypass, op1=ALU.mult, accum_out=n2[:, t : t + 1],
        )
    inv = small.tile([128, SUBT], F32, tag="inv")
    _emit_rsqrt(nc, small, n2[:], inv[:], SUBT)

    zn = work.tile([128, SUBT, D], BF16, tag="zn")
    for t in range(SUBT):
        nc.vector.tensor_scalar(
            zn[:, t], raw[:, t], inv[:, t : t + 1], None, op0=ALU.mult)

    # Store on the SWDGE (Pool) queue; transposes ride the sync HWDGE queue
    # so ScalarE's instruction stream stays free for the Exp hot loop.
    nc.gpsimd.dma_start(
        znorm_dram_ap.rearrange("(n p) d -> p n d", p=128), zn[:]
    )
    for k in range(KT):
        nc.sync.dma_start(
            out=zt_dst_aps[k],
            in_=znorm_dram_ap[:, k * 128 : (k + 1) * 128],
            transpose=True,
        )


def _emit_slab_raw(nc, pools, raw_src_ap, zraw_dram_ap, zt_dst_aps, invT_dst):
    """Slab (stationary) path: cast raw f32 -> bf16 (NO normalize -- 1/|z|
    folds into the Exp scale), stage + transpose; also produce
    invT_dst = INV_T / |z_row| [128, SUBT] for the activation scale."""
    work, small = pools["work"], pools["small"]
    raw = work.tile([128, SUBT, D], F32, tag="raw")
    nc.gpsimd.dma_start(raw[:], raw_src_ap)

    # Cast first: staging + transposes don't wait for the norm chain.
    zr = work.tile([128, SUBT, D], BF16, tag="zn")
    for t in range(SUBT):
        nc.vector.tensor_scalar(
            zr[:, t], raw[:, t], 1.0, None, op0=ALU.mult)
    nc.gpsimd.dma_start(
        zraw_dram_ap.rearrange("(n p) d -> p n d", p=128), zr[:]
    )
    for k in range(KT):
        nc.sync.dma_start(
            out=zt_dst_aps[k],
            in_=zraw_dram_ap[:, k * 128 : (k + 1) * 128],
            transpose=True,
        )

    sq_dump = work.tile([128, D], F32, tag="sqdump")
    n2 = small.tile([128, SUBT], F32, tag="n2")
    for t in range(SUBT):
        nc.vector.scalar_tensor_tensor(
            sq_dump[:], raw[:, t], 1.0, raw[:, t],
            op0=ALU.bypass, op1=ALU.mult, accum_out=n2[:, t : t + 1],
        )
    inv = small.tile([128, SUBT], F32, tag="inv")
    _emit_rsqrt(nc, small, n2[:], inv[:], SUBT)
    nc.vector.tensor_scalar(invT_dst, inv[:], INV_T, None, op0=ALU.mult)


def build_program(repeat=1):
    """repeat>1 re-emits the whole computation N times inside one NEFF —
    used only for steady-state timing (axon RPC latency swamps a single
    ~100us execution)."""
    nc = bacc.Bacc(
        "TRN2",
        target_bir_lowering=False,
        debug=False,
        num_devices=N_CORES,
    )
    z_i = nc.declare_dram_parameter("z_i", [B, D], F32, isOutput=False)
    z_j = nc.declare_dram_parameter("z_j", [B, D], F32, isOutput=False)
    z_slab = nc.declare_dram_parameter("z_slab", [SLAB, D], F32, isOutput=False)
    rowsums = nc.declare_dram_parameter("rowsums", [128, MT], F32, isOutput=True)

    zi_t = z_i.rearrange("(n p) d -> p n d", p=128)
    zj_t = z_j.rearrange("(n p) d -> p n d", p=128)
    zs_t = z_slab.rearrange("(n p) d -> p n d", p=128)

    with tile.TileContext(nc) as tc:
        with (
            tc.tile_pool(name="work", bufs=2) as work,
            tc.tile_pool(name="small", bufs=2) as small,
            tc.tile_pool(name="zt", bufs=1) as ztp,
            tc.tile_pool(name="dump", bufs=2) as dump,
            tc.tile_pool(name="psum", bufs=2, space="PSUM") as psum_pool,
            tc.tile_pool(name="dram", bufs=1, space="DRAM") as dram,
        ):
            pools = {"work": work, "small": small}

            # Warm the Exp activation table while DMAs run.
            warm = small.tile([128, 1], F32, tag="warm")
            nc.vector.memset(warm[:], 0.0)
            nc.scalar.activation(warm[:], warm[:], ACT.Exp)

            for _rep in range(repeat):
                # Persistent transposed embeddings (bf16, [K-part, k, row]).
                zts = ztp.tile([128, KT, SLAB], BF16, tag="zts", name="zts")
                ztn = [
                    ztp.tile(
                        [128, KT, SLAB], BF16, tag=f"ztn{g}", name=f"ztn{g}")
                    for g in range(GROUPS)
                ]
                invT = small.tile([128, SUBT], F32, tag="invT", name="invT")

                # Slab first: the stationary operand gates every matmul.
                zraw_dram = dram.tile(
                    [SLAB, D], BF16, tag="zslab_dram", name="zslab_dram")
                _emit_slab_raw(
                    nc, pools, zs_t[:, 0:SUBT],
                    zraw_dram[:],
                    [zts[:, k, :] for k in range(KT)],
                    invT[:],
                )
                # Full Z, one 1024-row group at a time.
                for g in range(GROUPS):
                    src = (
                        zi_t[:, g * SUBT : (g + 1) * SUBT]
                        if g < GROUPS // 2
                        else zj_t[
                            :,
                            (g - GROUPS // 2) * SUBT
                            : (g - GROUPS // 2 + 1) * SUBT,
                        ]
                    )
                    zn_dram = dram.tile(
                        [SLAB, D], BF16, tag=f"zn_dram{g}", name=f"zn_dram{g}")
                    _emit_normalize_group(
                        nc, pools, src, zn_dram[:],
                        [ztn[g][:, k, :] for k in range(KT)],
                    )

                # Main pass: slab x all-columns logits, exp, fused row sums.
                rsparts = small.tile(
                    [128, MT, NJG], F32, tag="rsparts", name="rsparts")
                for jg in range(NJG):
                    for m in range(MT):
                        ps = psum_pool.tile(
                            [128, JG * CHUNK], F32, tag="ps", name="ps")
                        # k outer / j inner: 4 consecutive matmuls share the
                        # stationary tile, so LDWEIGHTS hoists and the
                        # matmuls pipeline fill/drain back to back.
                        for k in range(KT):
                            for j in range(JG):
                                cidx = jg * JG + j
                                g, off = divmod(cidx * CHUNK, SLAB)
                                nc.tensor.matmul(
                                    ps[:, j * CHUNK : (j + 1) * CHUNK],
                                    zts[:, k, m * 128 : (m + 1) * 128],
                                    ztn[g][:, k, off : off + CHUNK],
                                    start=(k == 0),
                                    stop=(k == KT - 1),
                                )
                        ex = dump.tile(
                            [128, JG * CHUNK], BF16, tag="ex", name="ex")
                        nc.scalar.activation(
                            ex[:], ps[:], ACT.Exp,
                            scale=invT[:, m : m + 1],
                            accum_out=rsparts[:, m, jg : jg + 1],
                        )

                rs = small.tile([128, MT], F32, tag="rs", name="rs")
                nc.vector.tensor_reduce(
                    rs[:].rearrange("p (m o) -> p m o", o=1), rsparts[:],
                    axis=mybir.AxisListType.X, op=ALU.add,
                )
                nc.sync.dma_start(rowsums[:], rs[:])
    nc.compile()
    return nc


_PROGRAM = None


def _get_program():
    global _PROGRAM
    if _PROGRAM is None:
        _PROGRAM = build_program()
    return _PROGRAM


def run_device(z_i, z_j, **spmd_kwargs):
    """Run the SPMD kernel; returns ([N] row sums of exp(sim/T), raw results)."""
    nc = _get_program()
    z_all = np.concatenate([z_i, z_j], axis=0)
    in_maps = [
        {
            "z_i": z_i,
            "z_j": z_j,
            "z_slab": np.ascontiguousarray(z_all[c * SLAB : (c + 1) * SLAB]),
        }
        for c in range(N_CORES)
    ]
    out = run_bass_kernel_spmd(nc, in_maps, list(range(N_CORES)), **spmd_kwargs)
    rowsums = np.concatenate(
        [np.asarray(r["rowsums"]).T.reshape(SLAB) for r in out.results]
    )
    return rowsums, out


def finalize(z_i, z_j, rowsums):
    """Host-side O(N) finish: diagonal removal, log, positive-pair term."""
    rs = rowsums.astype(np.float64)
    lse = np.log(rs - math.exp(INV_T))          # drop masked diagonal exp(1/T)
    zi = z_i.astype(np.float64)
    zj = z_j.astype(np.float64)
    zi /= np.linalg.norm(zi, axis=1, keepdims=True)
    zj /= np.linalg.norm(zj, axis=1, keepdims=True)
    pos = np.sum(zi * zj)                       # = 0.5 * sum_r pos_r
    loss = (lse.sum() - 2.0 * pos * INV_T) / N
    return np.asarray(loss, dtype=np.float32)


def kernel(z_i, z_j):
    z_i = np.ascontiguousarray(np.asarray(z_i, dtype=np.float32))
    z_j = np.ascontiguousarray(np.asarray(z_j, dtype=np.float32))
    rowsums, _ = run_device(z_i, z_j)
    return finalize(z_i, z_j, rowsums)


if __name__ == "__main__":
    rng = np.random.default_rng(0)
    a = rng.standard_normal((B, D), dtype=np.float32)
    b = rng.standard_normal((B, D), dtype=np.float32)
    print(kernel(a, b))


# revision 4
# speedup vs baseline: 1.1602x; 1.1602x over previous
"""NT-Xent (SimCLR) contrastive loss on 8 Trainium2 NeuronCores.

Symmetric-matrix strategy: exp(sim/T) is symmetric, so each core computes
only 5/8 of its row-slab's columns and the missing 3/8 arrive as column
sums computed by other cores.

  Z = concat(z_i, z_j) -> [N=8192, D=256].  Core c receives z_rot =
  Z[c*1024 : c*1024+5*1024] (circularly) -- program group g = global slab
  (c+g)%8, so group 0 is always the core's own slab (the stationary
  operand; no redundant slab processing).  On device, each core
  - loads its 5 groups with a casting SWDGE DMA (f32 HBM -> bf16 SBUF),
    L2-normalizes them (DVE square+accum, bit-trick rsqrt, 1 Newton step),
    stages normalized bf16 rows to DRAM (scalar HWDGE) and xbar-transposes
    back (sync HWDGE) into ztn[g] = [128, k, 1024],
  - computes sim tiles vs its slab: diag block (d=0), then stripe
    [d=1|d=2], then stripe [d=3|d=4], k-outer/j-inner so matmuls share
    stationaries and pipeline; exp(sim/T) on ScalarE writes persistent
    ex_all bf16 tiles with fused row-sum accumulation (accum_out),
  - post-pass: ones-matmuls reduce ex_all columns for d in {1,2,3} into
    PSUM rows at partitions 0/32/64 (a freed PSUM rotation slot), giving
    the column sums that other slabs need (distances 5,6,7 by symmetry);
    d=4 blocks are computed row-only by both endpoint cores, so no column
    reduction is needed there,
  - DMAs out rowsums [128, 8] and colsums [97, 1024].
  The host combines row + column contributions, then computes
  loss = mean(log(S - e^{1/T}) - pos/T) in f64.
"""

import math

import numpy as np

import concourse.bacc as bacc
import concourse.bass as bass
import concourse.mybir as mybir
import concourse.tile as tile
from concourse.bass_utils import run_bass_kernel_spmd

B, D = 4096, 256
N = 2 * B                      # 8192 rows of Z
N_CORES = 8
SLAB = N // N_CORES            # 1024 rows per core
TEMPERATURE = 0.5
INV_T = 1.0 / TEMPERATURE      # 2.0

F32 = mybir.dt.float32
BF16 = mybir.dt.bfloat16
I32 = mybir.dt.int32
ALU = mybir.AluOpType
ACT = mybir.ActivationFunctionType

NG = 5                         # groups kept per core (d = 0..4)
SUBT = SLAB // 128             # 8 subtiles per group
KT = D // 128                  # 2 contraction tiles
CHUNK = 512                    # matmul moving free dim / PSUM bank
MT = SLAB // 128               # 8 output row tiles per core
EXW = NG * SLAB                # 5120 exp columns per row
NPARTS = 3                     # row-sum accumulators per m (T1 / T2 / diag)

RSQRT_MAGIC = 0x5F3759DF


def _emit_rsqrt(nc, pool, n2, inv, cols):
    """inv = 1/sqrt(n2) on DVE: quake seed + 1 Newton step (~0.2% max rel
    err -- far inside the 2e-2 loss tolerance)."""
    t_int = pool.tile([128, cols], I32, tag="rsq_i")
    y = pool.tile([128, cols], F32, tag="rsq_y")
    a = pool.tile([128, cols], F32, tag="rsq_a")
    c = pool.tile([128, cols], F32, tag="rsq_c")
    nc.vector.tensor_scalar(
        t_int[:], n2.bitcast(I32), 1, None, op0=ALU.logical_shift_right)
    nc.vector.tensor_scalar(
        y.bitcast(I32), t_int[:], -1, RSQRT_MAGIC, op0=ALU.mult, op1=ALU.add)
    nc.vector.scalar_tensor_tensor(
        a[:], y[:], 1.0, y[:], op0=ALU.bypass, op1=ALU.mult)
    nc.vector.scalar_tensor_tensor(
        c[:], a[:], -0.5, n2, op0=ALU.mult, op1=ALU.mult)
    nc.vector.scalar_tensor_tensor(
        inv, c[:], 1.5, y[:], op0=ALU.add, op1=ALU.mult)


def _emit_normalize_group(nc, pools, raw_src_ap, znorm_dram_ap, zt_dst_aps):
    """Casting-load 1024 raw rows (f32 HBM -> bf16 SBUF), L2-normalize,
    stage to DRAM, xbar-transpose back into [128, k, 1024] slices."""
    work, small = pools["work"], pools["small"]
    rawb = work.tile([128, SUBT, D], BF16, tag="rawb")
    nc.gpsimd.dma_start(rawb[:], raw_src_ap)   # SWDGE cast f32 -> bf16

    sq_dump = work.tile([128, D], BF16, tag="sqdump")
    n2 = small.tile([128, SUBT], F32, tag="n2")
    for t in range(SUBT):
        nc.vector.scalar_tensor_tensor(
            sq_dump[:], rawb[:, t], 1.0, rawb[:, t],
            op0=ALU.bypass, op1=ALU.mult, accum_out=n2[:, t : t + 1],
        )
    inv = small.tile([128, SUBT], F32, tag="inv")
    _emit_rsqrt(nc, small, n2[:], inv[:], SUBT)

    zn = work.tile([128, SUBT, D], BF16, tag="zn")
    for t in range(SUBT):
        nc.vector.tensor_scalar(
            zn[:, t], rawb[:, t], inv[:, t : t + 1], None, op0=ALU.mult)

    # Store on scalar HWDGE; transposes on sync HWDGE (separate rings, so
    # transposes never block the casting loads on gpsimd).
    nc.scalar.dma_start(
        znorm_dram_ap.rearrange("(n p) d -> p n d", p=128), zn[:]
    )
    for k in range(KT):
        nc.sync.dma_start(
            out=zt_dst_aps[k],
            in_=znorm_dram_ap[:, k * 128 : (k + 1) * 128],
            transpose=True,
        )


def build_program(repeat=1):
    nc = bacc.Bacc(
        "TRN2",
        target_bir_lowering=False,
        debug=False,
        num_devices=N_CORES,
    )
    z_rot = nc.declare_dram_parameter("z_rot", [NG * SLAB, D], F32,
                                      isOutput=False)
    rowsums = nc.declare_dram_parameter("rowsums", [128, MT], F32,
                                        isOutput=True)
    colsums = nc.declare_dram_parameter("colsums", [97, SLAB], F32,
                                        isOutput=True)

    zr_t = z_rot.rearrange("(n p) d -> p n d", p=128)

    with tile.TileContext(nc) as tc:
        with (
            tc.tile_pool(name="work", bufs=3) as work,
            tc.tile_pool(name="small", bufs=2) as small,
            tc.tile_pool(name="zt", bufs=1) as ztp,
            tc.tile_pool(name="ex", bufs=1) as exp_pool,
            tc.tile_pool(name="psum", bufs=2, space="PSUM") as psum_pool,
            tc.tile_pool(name="dram", bufs=1, space="DRAM") as dram,
        ):
            pools = {"work": work, "small": small}

            # Warm the Exp activation table while DMAs run; ones for the
            # column-sum matmuls.
            warm = small.tile([128, 1], F32, tag="warm")
            nc.vector.memset(warm[:], 0.0)
            nc.scalar.activation(warm[:], warm[:], ACT.Exp)
            ones = small.tile([128, 1], BF16, tag="ones")
            nc.vector.memset(ones[:], 1.0)

            for _rep in range(repeat):
                ztn = [
                    ztp.tile(
                        [128, KT, SLAB], BF16, tag=f"ztn{g}", name=f"ztn{g}")
                    for g in range(NG)
                ]
                # ex_all[p, m, 0:2048]=stripe d1|d2, [2048:4096]=d3|d4,
                # [4096:5120]=diag.
                ex_all = exp_pool.tile(
                    [128, MT, EXW], BF16, tag="ex_all", name="ex_all")
                rsparts = small.tile(
                    [128, MT, NPARTS], F32, tag="rsparts", name="rsparts")

                for g in range(NG):
                    zn_dram = dram.tile(
                        [SLAB, D], BF16, tag=f"zn_dram{g}", name=f"zn_dram{g}")
                    _emit_normalize_group(
                        nc, pools, zr_t[:, g * SUBT : (g + 1) * SUBT],
                        zn_dram[:],
                        [ztn[g][:, k, :] for k in range(KT)],
                    )

                # ---- main pass ----------------------------------------
                # Phase order tracks normalize completion: diag (g0 only),
                # stripe1 (g1, g2), stripe2 (g3, g4).
                def sim_tile(m, rhs_groups, ex_off, width, part):
                    ps = psum_pool.tile([128, 2 * CHUNK * 2], F32,
                                        tag="ps", name="ps")
                    nj = width // CHUNK
                    for k in range(KT):
                        for j in range(nj):
                            g = rhs_groups[(j * CHUNK) // SLAB]
                            off = (j * CHUNK) % SLAB
                            nc.tensor.matmul(
                                ps[:, j * CHUNK : (j + 1) * CHUNK],
                                ztn[0][:, k, m * 128 : (m + 1) * 128],
                                ztn[g][:, k, off : off + CHUNK],
                                start=(k == 0),
                                stop=(k == KT - 1),
                            )
                    nc.scalar.activation(
                        ex_all[:, m, ex_off : ex_off + width],
                        ps[:, 0:width], ACT.Exp, scale=INV_T,
                        accum_out=rsparts[:, m, part : part + 1],
                    )

                for m in range(MT):                      # diag: d=0
                    sim_tile(m, [0], 4096, SLAB, 0)
                for m in range(MT):                      # stripe: d=1,2
                    sim_tile(m, [1, 2], 0, 2 * SLAB, 1)
                for m in range(MT):                      # stripe: d=3,4
                    sim_tile(m, [3, 4], 2048, 2 * SLAB, 2)

                # ---- column sums (post-pass, overlaps last EXPs) ------
                # ones^T @ ex reduces over the 128 slab rows of each
                # m-tile; PSUM accumulates over m.  Rows land at
                # partitions 0/32/64 of a freed rotation slot.
                cs = psum_pool.tile([128, 2 * CHUNK * 2], F32,
                                    tag="ps", name="cs")
                for d in (1, 2, 3):
                    p0 = 32 * (d - 1)
                    for m in range(MT):
                        for h in range(2):   # N=512 halves: one PSUM bank each
                            nc.tensor.matmul(
                                cs[p0 : p0 + 1, h * CHUNK : (h + 1) * CHUNK],
                                ones[:, 0:1],
                                ex_all[:, m,
                                       (d - 1) * SLAB + h * CHUNK
                                       : (d - 1) * SLAB + (h + 1) * CHUNK],
                                start=(m == 0),
                                stop=(m == MT - 1),
                            )
                csb = small.tile([97, SLAB], F32, tag="csb", name="csb")
                nc.vector.tensor_copy(csb[:], cs[0:97, 0:SLAB])

                rs = small.tile([128, MT], F32, tag="rs", name="rs")
                nc.vector.tensor_reduce(
                    rs[:].rearrange("p (m o) -> p m o", o=1), rsparts[:],
                    axis=mybir.AxisListType.X, op=ALU.add,
                )
                nc.sync.dma_start(rowsums[:], rs[:])
                nc.sync.dma_start(colsums[:], csb[:])
    nc.compile()
    return nc


_PROGRAM = None


def _get_program():
    global _PROGRAM
    if _PROGRAM is None:
        _PROGRAM = build_program()
    return _PROGRAM


def run_device(z_i, z_j, **spmd_kwargs):
    """Run the SPMD kernel; returns ([N] row sums of exp(sim/T), results)."""
    nc = _get_program()
    z_all = np.concatenate([z_i, z_j], axis=0)
    z_ext = np.concatenate([z_all, z_all[: (NG - 1) * SLAB]], axis=0)
    in_maps = [
        {"z_rot": np.ascontiguousarray(
            z_ext[c * SLAB : c * SLAB + NG * SLAB])}
        for c in range(N_CORES)
    ]
    out = run_bass_kernel_spmd(nc, in_maps, list(range(N_CORES)), **spmd_kwargs)
    S = np.zeros(N, dtype=np.float64)
    for c in range(N_CORES):
        r = out.results[c]
        S[c * SLAB : (c + 1) * SLAB] += (
            np.asarray(r["rowsums"]).astype(np.float64).T.reshape(SLAB))
    for c in range(N_CORES):
        csb = np.asarray(out.results[c]["colsums"]).astype(np.float64)
        for d in (1, 2, 3):
            s = (c + d) % N_CORES
            S[s * SLAB : (s + 1) * SLAB] += csb[32 * (d - 1)]
    return S, out


def finalize(z_i, z_j, rowsums):
    """Host-side O(N) finish: diagonal removal, log, positive-pair term."""
    rs = rowsums.astype(np.float64)
    lse = np.log(rs - math.exp(INV_T))          # drop masked diagonal exp(1/T)
    zi = z_i.astype(np.float64)
    zj = z_j.astype(np.float64)
    zi /= np.linalg.norm(zi, axis=1, keepdims=True)
    zj /= np.linalg.norm(zj, axis=1, keepdims=True)
    pos = np.sum(zi * zj)                       # = 0.5 * sum_r pos_r
    loss = (lse.sum() - 2.0 * pos * INV_T) / N
    return np.asarray(loss, dtype=np.float32)


def kernel(z_i, z_j):
    z_i = np.ascontiguousarray(np.asarray(z_i, dtype=np.float32))
    z_j = np.ascontiguousarray(np.asarray(z_j, dtype=np.float32))
    rowsums, _ = run_device(z_i, z_j)
    return finalize(z_i, z_j, rowsums)


if __name__ == "__main__":
    rng = np.random.default_rng(0)
    a = rng.standard_normal((B, D), dtype=np.float32)
    b = rng.standard_normal((B, D), dtype=np.float32)
    print(kernel(a, b))


# revision 10
# speedup vs baseline: 1.1906x; 1.0262x over previous
"""NT-Xent (SimCLR) contrastive loss on 8 Trainium2 NeuronCores.

Symmetric-matrix strategy: exp(sim/T) is symmetric, so each core computes
only 5/8 of its row-slab's columns and the missing 3/8 arrive as column
sums computed by other cores.

  Z = concat(z_i, z_j) -> [N=8192, D=256].  Core c receives z_rot =
  Z[c*1024 : c*1024+5*1024] (circularly) -- program group g = global slab
  (c+g)%8, so group 0 is always the core's own slab (the stationary
  operand; no redundant slab processing).  On device, each core
  - loads its 5 groups with a casting SWDGE DMA (f32 HBM -> bf16 SBUF),
    L2-normalizes them (DVE square+accum, bit-trick rsqrt, 1 Newton step),
    stages normalized bf16 rows to DRAM (scalar HWDGE) and xbar-transposes
    back (sync HWDGE) into ztn[g] = [128, k, 1024],
  - computes sim tiles vs its slab: diag block (d=0), then stripe
    [d=1|d=2], then stripe [d=3|d=4], k-outer/j-inner so matmuls share
    stationaries and pipeline; exp(sim/T) on ScalarE writes persistent
    ex_all bf16 tiles with fused row-sum accumulation (accum_out),
  - post-pass: ones-matmuls reduce ex_all columns for d in {1,2,3} into
    PSUM rows at partitions 0/32/64 (a freed PSUM rotation slot), giving
    the column sums that other slabs need (distances 5,6,7 by symmetry);
    d=4 blocks are computed row-only by both endpoint cores, so no column
    reduction is needed there,
  - DMAs out rowsums [128, 8] and colsums [97, 1024].
  The host combines row + column contributions, then computes
  loss = mean(log(S - e^{1/T}) - pos/T) in f64.
"""

import math

import numpy as np

import concourse.bacc as bacc
import concourse.bass as bass
import concourse.mybir as mybir
import concourse.tile as tile
from concourse.bass_utils import run_bass_kernel_spmd

B, D = 4096, 256
N = 2 * B                      # 8192 rows of Z
N_CORES = 8
SLAB = N // N_CORES            # 1024 rows per core
TEMPERATURE = 0.5
INV_T = 1.0 / TEMPERATURE      # 2.0

F32 = mybir.dt.float32
BF16 = mybir.dt.bfloat16
I32 = mybir.dt.int32
ALU = mybir.AluOpType
ACT = mybir.ActivationFunctionType

NG = 5                         # groups kept per core (d = 0..4)
SUBT = SLAB // 128             # 8 subtiles per group
KT = D // 128                  # 2 contraction tiles
CHUNK = 512                    # matmul moving free dim / PSUM bank
MT = SLAB // 128               # 8 output row tiles per core
EXW = NG * SLAB                # 5120 exp columns per row
NPARTS = 3                     # row-sum accumulators per m (T1 / T2 / diag)

RSQRT_MAGIC = 0x5F3759DF


def _emit_rsqrt(nc, pool, n2, inv, cols):
    """inv = 1/sqrt(n2) on DVE: quake seed + 1 Newton step (~0.2% max rel
    err -- far inside the 2e-2 loss tolerance)."""
    t_int = pool.tile([128, cols], I32, tag="rsq_i")
    y = pool.tile([128, cols], F32, tag="rsq_y")
    a = pool.tile([128, cols], F32, tag="rsq_a")
    c = pool.tile([128, cols], F32, tag="rsq_c")
    nc.vector.tensor_scalar(
        t_int[:], n2.bitcast(I32), 1, None, op0=ALU.logical_shift_right)
    nc.vector.tensor_scalar(
        y.bitcast(I32), t_int[:], -1, RSQRT_MAGIC, op0=ALU.mult, op1=ALU.add)
    nc.vector.scalar_tensor_tensor(
        a[:], y[:], 1.0, y[:], op0=ALU.bypass, op1=ALU.mult)
    nc.vector.scalar_tensor_tensor(
        c[:], a[:], -0.5, n2, op0=ALU.mult, op1=ALU.mult)
    nc.vector.scalar_tensor_tensor(
        inv, c[:], 1.5, y[:], op0=ALU.add, op1=ALU.mult)


def _emit_normalize_group(nc, pools, raw_src_ap, znorm_dram_ap, zt_dst_aps,
                          nt=SUBT, warm_mm=None):
    """Load 1024 raw f32 rows, L2-normalize them (bf16 out), stage to DRAM,
    xbar-transpose back into [128, k, 1024] slices.

    Queues: loads ride sync HWDGE (fast start + fast completion, so
    round-robin DMA-sem lane recycling never blocks the transposes on a
    slow load), stores ride gpsimd SWDGE, transposes ride scalar HWDGE."""
    work, small = pools["work"], pools["small"]
    raw = work.tile([128, nt, D], F32, tag=f"raw{nt}")
    nc.sync.dma_start(raw[:], raw_src_ap)

    sq_dump = work.tile([128, D], BF16, tag="sqdump")
    n2 = small.tile([128, nt], F32, tag=f"n2_{nt}")
    for t in range(nt):
        nc.vector.scalar_tensor_tensor(
            sq_dump[:], raw[:, t], 1.0, raw[:, t],
            op0=ALU.bypass, op1=ALU.mult, accum_out=n2[:, t : t + 1],
        )
    inv = small.tile([128, nt], F32, tag=f"inv{nt}")
    _emit_rsqrt(nc, small, n2[:], inv[:], nt)

    zn = work.tile([128, nt, D], BF16, tag=f"zn{nt}")
    for t in range(nt):
        nc.vector.tensor_scalar(
            zn[:, t], raw[:, t], inv[:, t : t + 1], None, op0=ALU.mult)
        if warm_mm is not None:
            # Keep the PE's HAM activity window busy through the ramp so
            # the first real matmuls run at 2.4 GHz instead of 1.2.
            for _ in range(2):
                nc.tensor.matmul(
                    warm_mm[0:64, 0:256], zn[:, t, 0:64], zn[:, t, :],
                    start=True, stop=True,
                )

    nc.gpsimd.dma_start(
        znorm_dram_ap.rearrange("(n p) d -> p n d", p=128), zn[:]
    )
    for k in range(KT):
        nc.scalar.dma_start(
            out=zt_dst_aps[k],
            in_=znorm_dram_ap[:, k * 128 : (k + 1) * 128],
            transpose=True,
        )


def build_program(repeat=1):
    nc = bacc.Bacc(
        "TRN2",
        target_bir_lowering=False,
        debug=False,
        num_devices=N_CORES,
    )
    z_rot = nc.declare_dram_parameter("z_rot", [NG * SLAB, D], F32,
                                      isOutput=False)
    rowsums = nc.declare_dram_parameter("rowsums", [128, MT], F32,
                                        isOutput=True)
    colsums = nc.declare_dram_parameter("colsums", [97, SLAB], F32,
                                        isOutput=True)

    zr_t = z_rot.rearrange("(n p) d -> p n d", p=128)

    with tile.TileContext(nc) as tc:
        with (
            tc.tile_pool(name="work", bufs=3) as work,
            tc.tile_pool(name="small", bufs=2) as small,
            tc.tile_pool(name="zt", bufs=1) as ztp,
            tc.tile_pool(name="ex", bufs=1) as exp_pool,
            tc.tile_pool(name="psum", bufs=2, space="PSUM") as psum_pool,
            tc.tile_pool(name="dram", bufs=1, space="DRAM") as dram,
        ):
            pools = {"work": work, "small": small}

            # Warm the Exp activation table while DMAs run; ones for the
            # column-sum matmuls.
            warm = small.tile([128, 1], F32, tag="warm")
            nc.vector.memset(warm[:], 0.0)
            nc.scalar.activation(warm[:], warm[:], ACT.Exp)
            ones = small.tile([128, 1], BF16, tag="ones")
            nc.vector.memset(ones[:], 1.0)

            for _rep in range(repeat):
                ztn = [
                    ztp.tile(
                        [128, KT, SLAB], BF16, tag=f"ztn{g}", name=f"ztn{g}")
                    for g in range(NG)
                ]
                # ex_all[p, m, 0:2048]=stripe d1|d2, [2048:4096]=d3|d4,
                # [4096:5120]=diag.
                ex_all = exp_pool.tile(
                    [128, MT, EXW], BF16, tag="ex_all", name="ex_all")
                rsparts = small.tile(
                    [128, MT, NPARTS], F32, tag="rsparts", name="rsparts")

                warm_ps = psum_pool.tile([128, 2 * CHUNK * 2], F32,
                                         tag="ps", name="warm_ps")
                # Group 0 (the slab -- it gates every matmul) is processed
                # as two 512-row halves so load/normalize/stage/transpose
                # pipeline against each other.
                HS = SUBT // 2
                for h in range(2):
                    zn_dram = dram.tile(
                        [SLAB // 2, D], BF16, tag=f"zn_dram0{h}",
                        name=f"zn_dram0{h}")
                    _emit_normalize_group(
                        nc, pools, zr_t[:, h * HS : (h + 1) * HS],
                        zn_dram[:],
                        [ztn[0][:, k, h * 512 : (h + 1) * 512]
                         for k in range(KT)],
                        nt=HS,
                        warm_mm=warm_ps if h == 1 else None,
                    )
                for g in range(1, NG):
                    zn_dram = dram.tile(
                        [SLAB, D], BF16, tag=f"zn_dram{g}", name=f"zn_dram{g}")
                    _emit_normalize_group(
                        nc, pools, zr_t[:, g * SUBT : (g + 1) * SUBT],
                        zn_dram[:],
                        [ztn[g][:, k, :] for k in range(KT)],
                    )
                # Private copy of the slab for the diag tiles' moving
                # operand: stationary and moving reading the same SBUF
                # tile halves matmul throughput (read-port conflict).
                ztn0m = ztp.tile([128, KT, SLAB], BF16, tag="ztn0m",
                                 name="ztn0m")
                nc.vector.tensor_copy(ztn0m[:], ztn[0][:])

                # ---- main pass ----------------------------------------
                # Phase order tracks normalize completion: diag (g0 only),
                # stripe1 (g1, g2), stripe2 (g3, g4).
                def sim_tile(m, rhs_tiles, ex_off, width, part):
                    ps = psum_pool.tile([128, 2 * CHUNK * 2], F32,
                                        tag="ps", name="ps")
                    nj = width // CHUNK
                    for k in range(KT):
                        for j in range(nj):
                            rt = rhs_tiles[(j * CHUNK) // SLAB]
                            off = (j * CHUNK) % SLAB
                            nc.tensor.matmul(
                                ps[:, j * CHUNK : (j + 1) * CHUNK],
                                ztn[0][:, k, m * 128 : (m + 1) * 128],
                                rt[:, k, off : off + CHUNK],
                                start=(k == 0),
                                stop=(k == KT - 1),
                            )
                    nc.scalar.activation(
                        ex_all[:, m, ex_off : ex_off + width],
                        ps[:, 0:width], ACT.Exp, scale=INV_T,
                        accum_out=rsparts[:, m, part : part + 1],
                    )

                for m in range(MT):                      # diag: d=0
                    sim_tile(m, [ztn0m], 4096, SLAB, 0)
                for m in range(MT):                      # stripe: d=1,2
                    sim_tile(m, [ztn[1], ztn[2]], 0, 2 * SLAB, 1)
                for m in range(MT):                      # stripe: d=3,4
                    sim_tile(m, [ztn[3], ztn[4]], 2048, 2 * SLAB, 2)

                # ---- column sums (post-pass, overlaps last EXPs) ------
                # ones^T @ ex reduces over the 128 slab rows of each
                # m-tile; PSUM accumulates over m.  Rows land at
                # partitions 0/32/64 of a freed rotation slot.
                cs = psum_pool.tile([128, 2 * CHUNK * 2], F32,
                                    tag="ps", name="cs")
                for d in (1, 2, 3):
                    p0 = 32 * (d - 1)
                    for m in range(MT):
                        for h in range(2):   # N=512 halves: one PSUM bank each
                            nc.tensor.matmul(
                                cs[p0 : p0 + 1, h * CHUNK : (h + 1) * CHUNK],
                                ones[:, 0:1],
                                ex_all[:, m,
                                       (d - 1) * SLAB + h * CHUNK
                                       : (d - 1) * SLAB + (h + 1) * CHUNK],
                                start=(m == 0),
                                stop=(m == MT - 1),
                            )
                csb = small.tile([97, SLAB], F32, tag="csb", name="csb")
                nc.vector.tensor_copy(csb[:], cs[0:97, 0:SLAB])

                rs = small.tile([128, MT], F32, tag="rs", name="rs")
                nc.vector.tensor_reduce(
                    rs[:].rearrange("p (m o) -> p m o", o=1), rsparts[:],
                    axis=mybir.AxisListType.X, op=ALU.add,
                )
                nc.sync.dma_start(rowsums[:], rs[:])
                nc.sync.dma_start(colsums[:], csb[:])
    nc.compile()
    return nc


_PROGRAM = None


def _get_program():
    global _PROGRAM
    if _PROGRAM is None:
        _PROGRAM = build_program()
    return _PROGRAM


def run_device(z_i, z_j, **spmd_kwargs):
    """Run the SPMD kernel; returns ([N] row sums of exp(sim/T), results)."""
    nc = _get_program()
    z_all = np.concatenate([z_i, z_j], axis=0)
    z_ext = np.concatenate([z_all, z_all[: (NG - 1) * SLAB]], axis=0)
    in_maps = [
        {"z_rot": np.ascontiguousarray(
            z_ext[c * SLAB : c * SLAB + NG * SLAB])}
        for c in range(N_CORES)
    ]
    out = run_bass_kernel_spmd(nc, in_maps, list(range(N_CORES)), **spmd_kwargs)
    S = np.zeros(N, dtype=np.float64)
    for c in range(N_CORES):
        r = out.results[c]
        S[c * SLAB : (c + 1) * SLAB] += (
            np.asarray(r["rowsums"]).astype(np.float64).T.reshape(SLAB))
    for c in range(N_CORES):
        csb = np.asarray(out.results[c]["colsums"]).astype(np.float64)
        for d in (1, 2, 3):
            s = (c + d) % N_CORES
            S[s * SLAB : (s + 1) * SLAB] += csb[32 * (d - 1)]
    return S, out


def finalize(z_i, z_j, rowsums):
    """Host-side O(N) finish: diagonal removal, log, positive-pair term."""
    rs = rowsums.astype(np.float64)
    lse = np.log(rs - math.exp(INV_T))          # drop masked diagonal exp(1/T)
    zi = z_i.astype(np.float64)
    zj = z_j.astype(np.float64)
    zi /= np.linalg.norm(zi, axis=1, keepdims=True)
    zj /= np.linalg.norm(zj, axis=1, keepdims=True)
    pos = np.sum(zi * zj)                       # = 0.5 * sum_r pos_r
    loss = (lse.sum() - 2.0 * pos * INV_T) / N
    return np.asarray(loss, dtype=np.float32)


def kernel(z_i, z_j):
    z_i = np.ascontiguousarray(np.asarray(z_i, dtype=np.float32))
    z_j = np.ascontiguousarray(np.asarray(z_j, dtype=np.float32))
    rowsums, _ = run_device(z_i, z_j)
    return finalize(z_i, z_j, rowsums)


if __name__ == "__main__":
    rng = np.random.default_rng(0)
    a = rng.standard_normal((B, D), dtype=np.float32)
    b = rng.standard_normal((B, D), dtype=np.float32)
    print(kernel(a, b))


# revision 13
# speedup vs baseline: 1.4248x; 1.1967x over previous
"""NT-Xent (SimCLR) contrastive loss on 8 Trainium2 NeuronCores.

Symmetric-matrix strategy: exp(sim/T) is symmetric, so each core computes
only 5/8 of its row-slab's columns and the missing 3/8 arrive as column
sums computed by other cores.

  Z = concat(z_i, z_j) -> [N=8192, D=256].  Core c receives z_rot =
  Z[c*1024 : c*1024+5*1024] (circularly) -- program group g = global slab
  (c+g)%8, so group 0 is always the core's own slab (the stationary
  operand; no redundant slab processing).  On device, each core
  - loads its 5 groups with a casting SWDGE DMA (f32 HBM -> bf16 SBUF),
    L2-normalizes them (DVE square+accum, bit-trick rsqrt, 1 Newton step),
    stages normalized bf16 rows to DRAM (scalar HWDGE) and xbar-transposes
    back (sync HWDGE) into ztn[g] = [128, k, 1024],
  - computes sim tiles vs its slab: diag block (d=0), then stripe
    [d=1|d=2], then stripe [d=3|d=4], k-outer/j-inner so matmuls share
    stationaries and pipeline; exp(sim/T) on ScalarE writes persistent
    ex_all bf16 tiles with fused row-sum accumulation (accum_out),
  - post-pass: ones-matmuls reduce ex_all columns for d in {1,2,3} into
    PSUM rows at partitions 0/32/64 (a freed PSUM rotation slot), giving
    the column sums that other slabs need (distances 5,6,7 by symmetry);
    d=4 blocks are computed row-only by both endpoint cores, so no column
    reduction is needed there,
  - DMAs out rowsums [128, 8] and colsums [97, 1024].
  The host combines row + column contributions, then computes
  loss = mean(log(S - e^{1/T}) - pos/T) in f64.
"""

import math

import numpy as np

import concourse.bacc as bacc
import concourse.bass as bass
import concourse.mybir as mybir
import concourse.tile as tile
from concourse.bass_utils import run_bass_kernel_spmd

B, D = 4096, 256
N = 2 * B                      # 8192 rows of Z
N_CORES = 8
SLAB = N // N_CORES            # 1024 rows per core
TEMPERATURE = 0.5
INV_T = 1.0 / TEMPERATURE      # 2.0

F32 = mybir.dt.float32
BF16 = mybir.dt.bfloat16
I32 = mybir.dt.int32
ALU = mybir.AluOpType
ACT = mybir.ActivationFunctionType

NG = 5                         # groups kept per core (d = 0..4)
SUBT = SLAB // 128             # 8 subtiles per group
KT = D // 128                  # 2 contraction tiles
CHUNK = 512                    # matmul moving free dim / PSUM bank
MT = SLAB // 128               # 8 output row tiles per core
EXW = NG * SLAB                # 5120 exp columns per row
NPARTS = 3                     # row-sum accumulators per m (T1 / T2 / diag)

RSQRT_MAGIC = 0x5F3759DF


def _emit_rsqrt(nc, pool, n2, inv, cols):
    """inv = 1/sqrt(n2) on DVE: quake seed + 1 Newton step (~0.2% max rel
    err -- far inside the 2e-2 loss tolerance)."""
    t_int = pool.tile([128, cols], I32, tag="rsq_i")
    y = pool.tile([128, cols], F32, tag="rsq_y")
    a = pool.tile([128, cols], F32, tag="rsq_a")
    c = pool.tile([128, cols], F32, tag="rsq_c")
    nc.vector.tensor_scalar(
        t_int[:], n2.bitcast(I32), 1, None, op0=ALU.logical_shift_right)
    nc.vector.tensor_scalar(
        y.bitcast(I32), t_int[:], -1, RSQRT_MAGIC, op0=ALU.mult, op1=ALU.add)
    nc.vector.scalar_tensor_tensor(
        a[:], y[:], 1.0, y[:], op0=ALU.bypass, op1=ALU.mult)
    nc.vector.scalar_tensor_tensor(
        c[:], a[:], -0.5, n2, op0=ALU.mult, op1=ALU.mult)
    nc.vector.scalar_tensor_tensor(
        inv, c[:], 1.5, y[:], op0=ALU.add, op1=ALU.mult)


def _emit_normalize_group(nc, pools, raw_src_ap, znorm_dram_ap, zt_dst_aps,
                          nt=SUBT, warm_mm=None):
    """Load 1024 raw f32 rows, L2-normalize them (bf16 out), stage to DRAM,
    xbar-transpose back into [128, k, 1024] slices.

    Queues: loads ride sync HWDGE (fast start + fast completion, so
    round-robin DMA-sem lane recycling never blocks the transposes on a
    slow load), stores ride gpsimd SWDGE, transposes ride scalar HWDGE."""
    work, small = pools["work"], pools["small"]
    raw = work.tile([128, nt, D], F32, tag=f"raw{nt}")
    nc.sync.dma_start(raw[:], raw_src_ap)

    sq_dump = work.tile([128, D], BF16, tag="sqdump")
    n2 = small.tile([128, nt], F32, tag=f"n2_{nt}")
    for t in range(nt):
        nc.vector.scalar_tensor_tensor(
            sq_dump[:], raw[:, t], 1.0, raw[:, t],
            op0=ALU.bypass, op1=ALU.mult, accum_out=n2[:, t : t + 1],
        )
    inv = small.tile([128, nt], F32, tag=f"inv{nt}")
    _emit_rsqrt(nc, small, n2[:], inv[:], nt)

    zn = work.tile([128, nt, D], BF16, tag=f"zn{nt}")
    for t in range(nt):
        nc.vector.tensor_scalar(
            zn[:, t], raw[:, t], inv[:, t : t + 1], None, op0=ALU.mult)
        if warm_mm is not None:
            # Keep the PE's HAM activity window busy through the ramp so
            # the first real matmuls run at 2.4 GHz instead of 1.2 (the
            # un-throttle needs ~3.4us of sustained PE activity).
            for _ in range(4):
                nc.tensor.matmul(
                    warm_mm[0:64, 0:256], zn[:, t, 0:64], zn[:, t, :],
                    start=True, stop=True,
                )

    nc.gpsimd.dma_start(
        znorm_dram_ap.rearrange("(n p) d -> p n d", p=128), zn[:]
    )
    for k in range(KT):
        nc.sync.dma_start(
            out=zt_dst_aps[k],
            in_=znorm_dram_ap[:, k * 128 : (k + 1) * 128],
            transpose=True,
        )


def build_program(repeat=1):
    nc = bacc.Bacc(
        "TRN2",
        target_bir_lowering=False,
        debug=False,
        num_devices=N_CORES,
    )
    z_rot = nc.declare_dram_parameter("z_rot", [NG * SLAB, D], F32,
                                      isOutput=False)
    rowsums = nc.declare_dram_parameter("rowsums", [128, MT], F32,
                                        isOutput=True)
    colsums = nc.declare_dram_parameter("colsums", [97, SLAB], F32,
                                        isOutput=True)

    zr_t = z_rot.rearrange("(n p) d -> p n d", p=128)

    with tile.TileContext(nc) as tc:
        with (
            tc.tile_pool(name="work", bufs=3) as work,
            tc.tile_pool(name="small", bufs=2) as small,
            tc.tile_pool(name="zt", bufs=1) as ztp,
            tc.tile_pool(name="ex", bufs=1) as exp_pool,
            tc.tile_pool(name="psum", bufs=2, space="PSUM") as psum_pool,
            tc.tile_pool(name="dram", bufs=1, space="DRAM") as dram,
        ):
            pools = {"work": work, "small": small}

            # Warm the Exp activation table while DMAs run; ones for the
            # column-sum matmuls.
            warm = small.tile([128, 1], F32, tag="warm")
            nc.vector.memset(warm[:], 0.0)
            nc.scalar.activation(warm[:], warm[:], ACT.Exp)
            ones = small.tile([128, 1], BF16, tag="ones")
            nc.vector.memset(ones[:], 1.0)

            for _rep in range(repeat):
                ztn = [
                    ztp.tile(
                        [128, KT, SLAB], BF16, tag=f"ztn{g}", name=f"ztn{g}")
                    for g in range(NG)
                ]
                # ex_all[p, m, 0:2048]=stripe d1|d2, [2048:4096]=d3|d4,
                # [4096:5120]=diag.
                ex_all = exp_pool.tile(
                    [128, MT, EXW], BF16, tag="ex_all", name="ex_all")
                rsparts = small.tile(
                    [128, MT, NPARTS], F32, tag="rsparts", name="rsparts")

                warm_ps = psum_pool.tile([128, 2 * CHUNK * 2], F32,
                                         tag="ps", name="warm_ps")
                # Group 0 (the slab -- it gates every matmul) is processed
                # as two 512-row halves so load/normalize/stage/transpose
                # pipeline against each other.
                HS = SUBT // 2
                for h in range(2):
                    zn_dram = dram.tile(
                        [SLAB // 2, D], BF16, tag=f"zn_dram0{h}",
                        name=f"zn_dram0{h}")
                    _emit_normalize_group(
                        nc, pools, zr_t[:, h * HS : (h + 1) * HS],
                        zn_dram[:],
                        [ztn[0][:, k, h * 512 : (h + 1) * 512]
                         for k in range(KT)],
                        nt=HS,
                        warm_mm=warm_ps,
                    )
                for g in range(1, NG):
                    zn_dram = dram.tile(
                        [SLAB, D], BF16, tag=f"zn_dram{g}", name=f"zn_dram{g}")
                    _emit_normalize_group(
                        nc, pools, zr_t[:, g * SUBT : (g + 1) * SUBT],
                        zn_dram[:],
                        [ztn[g][:, k, :] for k in range(KT)],
                    )
                # Private copy of the slab for the diag tiles' moving
                # operand: stationary and moving reading the same SBUF
                # tile halves matmul throughput (read-port conflict).
                ztn0m = ztp.tile([128, KT, SLAB], BF16, tag="ztn0m",
                                 name="ztn0m")
                nc.vector.tensor_copy(ztn0m[:], ztn[0][:])

                # ---- main pass ----------------------------------------
                # Phase order tracks normalize completion: diag (g0 only),
                # stripe1 (g1, g2), stripe2 (g3, g4).
                def sim_tile(m, rhs_tiles, ex_off, width, part):
                    ps = psum_pool.tile([128, 2 * CHUNK * 2], F32,
                                        tag="ps", name="ps")
                    nj = width // CHUNK
                    for k in range(KT):
                        for j in range(nj):
                            rt = rhs_tiles[(j * CHUNK) // SLAB]
                            off = (j * CHUNK) % SLAB
                            nc.tensor.matmul(
                                ps[:, j * CHUNK : (j + 1) * CHUNK],
                                ztn[0][:, k, m * 128 : (m + 1) * 128],
                                rt[:, k, off : off + CHUNK],
                                start=(k == 0),
                                stop=(k == KT - 1),
                            )
                    nc.scalar.activation(
                        ex_all[:, m, ex_off : ex_off + width],
                        ps[:, 0:width], ACT.Exp, scale=INV_T,
                        accum_out=rsparts[:, m, part : part + 1],
                    )

                for m in range(MT):                      # diag: d=0
                    sim_tile(m, [ztn0m], 4096, SLAB, 0)
                for m in range(MT):                      # stripe: d=1,2
                    sim_tile(m, [ztn[1], ztn[2]], 0, 2 * SLAB, 1)
                for m in range(MT):                      # stripe: d=3,4
                    sim_tile(m, [ztn[3], ztn[4]], 2048, 2 * SLAB, 2)

                # ---- column sums (post-pass, overlaps last EXPs) ------
                # ones^T @ ex reduces over the 128 slab rows of each
                # m-tile; PSUM accumulates over m.  Rows land at
                # partitions 0/32/64 of a freed rotation slot.
                cs = psum_pool.tile([128, 2 * CHUNK * 2], F32,
                                    tag="ps", name="cs")
                for d in (1, 2, 3):
                    p0 = 32 * (d - 1)
                    for m in range(MT):
                        for h in range(2):   # N=512 halves: one PSUM bank each
                            nc.tensor.matmul(
                                cs[p0 : p0 + 1, h * CHUNK : (h + 1) * CHUNK],
                                ones[:, 0:1],
                                ex_all[:, m,
                                       (d - 1) * SLAB + h * CHUNK
                                       : (d - 1) * SLAB + (h + 1) * CHUNK],
                                start=(m == 0),
                                stop=(m == MT - 1),
                            )
                csb = small.tile([97, SLAB], F32, tag="csb", name="csb")
                nc.vector.tensor_copy(csb[:], cs[0:97, 0:SLAB])

                rs = small.tile([128, MT], F32, tag="rs", name="rs")
                nc.vector.tensor_reduce(
                    rs[:].rearrange("p (m o) -> p m o", o=1), rsparts[:],
                    axis=mybir.AxisListType.X, op=ALU.add,
                )
                nc.sync.dma_start(rowsums[:], rs[:])
                nc.sync.dma_start(colsums[:], csb[:])
    nc.compile()
    return nc


_PROGRAM = None


def _get_program():
    global _PROGRAM
    if _PROGRAM is None:
        _PROGRAM = build_program()
    return _PROGRAM


def run_device(z_i, z_j, **spmd_kwargs):
    """Run the SPMD kernel; returns ([N] row sums of exp(sim/T), results)."""
    nc = _get_program()
    z_all = np.concatenate([z_i, z_j], axis=0)
    z_ext = np.concatenate([z_all, z_all[: (NG - 1) * SLAB]], axis=0)
    in_maps = [
        {"z_rot": np.ascontiguousarray(
            z_ext[c * SLAB : c * SLAB + NG * SLAB])}
        for c in range(N_CORES)
    ]
    out = run_bass_kernel_spmd(nc, in_maps, list(range(N_CORES)), **spmd_kwargs)
    S = np.zeros(N, dtype=np.float64)
    for c in range(N_CORES):
        r = out.results[c]
        S[c * SLAB : (c + 1) * SLAB] += (
            np.asarray(r["rowsums"]).astype(np.float64).T.reshape(SLAB))
    for c in range(N_CORES):
        csb = np.asarray(out.results[c]["colsums"]).astype(np.float64)
        for d in (1, 2, 3):
            s = (c + d) % N_CORES
            S[s * SLAB : (s + 1) * SLAB] += csb[32 * (d - 1)]
    return S, out


def finalize(z_i, z_j, rowsums):
    """Host-side O(N) finish: diagonal removal, log, positive-pair term."""
    rs = rowsums.astype(np.float64)
    lse = np.log(rs - math.exp(INV_T))          # drop masked diagonal exp(1/T)
    zi = z_i.astype(np.float64)
    zj = z_j.astype(np.float64)
    zi /= np.linalg.norm(zi, axis=1, keepdims=True)
    zj /= np.linalg.norm(zj, axis=1, keepdims=True)
    pos = np.sum(zi * zj)                       # = 0.5 * sum_r pos_r
    loss = (lse.sum() - 2.0 * pos * INV_T) / N
    return np.asarray(loss, dtype=np.float32)


def kernel(z_i, z_j):
    z_i = np.ascontiguousarray(np.asarray(z_i, dtype=np.float32))
    z_j = np.ascontiguousarray(np.asarray(z_j, dtype=np.float32))
    rowsums, _ = run_device(z_i, z_j)
    return finalize(z_i, z_j, rowsums)


if __name__ == "__main__":
    rng = np.random.default_rng(0)
    a = rng.standard_normal((B, D), dtype=np.float32)
    b = rng.standard_normal((B, D), dtype=np.float32)
    print(kernel(a, b))
